# revision 8
# baseline (speedup 1.0000x reference)
"""Gated TCN layer (fully conditioned) as a Bass/Tile kernel on 8 NeuronCores.

Reference computation (per sample b):
    kern = (c @ adapter_w + adapter_b).reshape(2*CH, CH, K)
    y    = dilated causal conv of x with per-sample kern (K=3, dil=4)
    y   += (c @ bias_w + bias_b)[:, None]
    z    = tanh(y[:CH]) * sigmoid(y[CH:])
    out  = resi_w @ z + resi_b + x
Returns (out, z).

Sharding: data-parallel over batch, 2 samples per core. The two samples are
packed on the 128 SBUF partitions (rows 64b+ch) so every activation / vector
op runs full-width. The per-sample conv kernels are laid out block-diagonally
per (gate, tap) so one matmul computes one gate half for both samples at once.
All matmul operands are bf16 (full PE rate + FWL); accumulation is fp32 PSUM.
"""

import numpy as np

from concourse import bacc, mybir, tile
from concourse.bass_utils import run_bass_kernel_spmd

K = 3
DIL = 4
CH = 64
COND = 128
B, T = 16, 16384
NCORES = 8
BL = B // NCORES          # samples per core
PAD = (K - 1) * DIL       # causal left pad = 8
NT = 512                  # matmul free-dim (one PSUM bank of fp32)
UW = 1024                 # processing unit width (2 PSUM banks)
NJ = T // UW
F = K * CH * 2 * CH       # 24576 adapter columns
QCH = CH * CH             # 4096 adapter columns per (gate, tap) block
NXC = 4                   # x load chunks
QLIST = [0, 2, 4, 1, 3, 5]  # stream tanh-gate (even q) blocks first

F32 = mybir.dt.float32
BF16 = mybir.dt.bfloat16
AF = mybir.ActivationFunctionType
ALU = mybir.AluOpType

# Set by test.py to capture a profile; harness path leaves these alone.
TRACE = False
LAST_RESULTS = None

_NC = None


def _build():
    nc = bacc.Bacc("TRN2", target_bir_lowering=False, debug=False)

    x_in = nc.dram_tensor("x_in", [2 * CH, T], BF16, kind="ExternalInput")
    cT_d = nc.dram_tensor("cT", [COND, BL], BF16, kind="ExternalInput")
    aw_d = nc.dram_tensor("aw_r", [COND, F], BF16, kind="ExternalInput")
    ab_d = nc.dram_tensor("ab_p", [2 * CH, 6 * 128], BF16, kind="ExternalInput")
    bw_d = nc.dram_tensor("bw", [COND, 2 * CH], BF16, kind="ExternalInput")
    bbt_d = nc.dram_tensor("bbt", [2 * CH, 1], F32, kind="ExternalInput")
    bbs_d = nc.dram_tensor("bbs", [2 * CH, 1], F32, kind="ExternalInput")
    rwT_d = nc.dram_tensor("rwT", [2 * CH, 2 * CH], BF16, kind="ExternalInput")
    rb_d = nc.dram_tensor("rb", [2 * CH, 1], F32, kind="ExternalInput")
    out_d = nc.dram_tensor("out_d", [2 * CH, T], BF16, kind="ExternalOutput")
    z_d = nc.dram_tensor("z_d", [2 * CH, T], BF16, kind="ExternalOutput")

    with tile.TileContext(nc) as tc:
        with (
            tc.tile_pool(name="const", bufs=1) as constp,
            tc.tile_pool(name="xpool", bufs=1) as xpool,
            tc.tile_pool(name="kern", bufs=1) as kernp,
        ):
            cT_sb = constp.tile([COND, BL], BF16)
            nc.sync.dma_start(cT_sb[:, :], cT_d[:, :])
            bw_sb = constp.tile([COND, 2 * CH], BF16)
            nc.sync.dma_start(bw_sb[:, :], bw_d[:, :])

            # Adapter weight chunks: all 6 DMA triggers issued upfront so the
            # loads run concurrently (higher aggregate HBM bandwidth).
            awts = []
            for cidx in range(6):
                awt = constp.tile([COND, QCH], BF16, name=f"awt{cidx}", tag=f"aw{cidx}")
                nc.sync.dma_start(awt[:, :], aw_d[:, cidx * QCH : (cidx + 1) * QCH])
                awts.append(awt)

            # x for both samples packed on 128 partitions, left-padded by PAD.
            # Small chunks first so the tanh conv pass can start early.
            xz = xpool.tile([2 * CH, PAD + T], BF16)
            nc.vector.memset(xz[:, 0:PAD].bitcast(F32), 0.0)
            xc_sizes = [2048, 2048, 4096, 8192]
            c0x = 0
            for sz in xc_sizes:
                nc.sync.dma_start(
                    xz[:, PAD + c0x : PAD + c0x + sz], x_in[:, c0x : c0x + sz]
                )
                c0x += sz

            ab_sb = constp.tile([2 * CH, 6 * 128], BF16)
            nc.sync.dma_start(ab_sb[:, :], ab_d[:, :])
            rwT_sb = constp.tile([2 * CH, 2 * CH], BF16)
            nc.sync.dma_start(rwT_sb[:, :], rwT_d[:, :])
            rb_sb = constp.tile([2 * CH, 1], F32)
            nc.sync.dma_start(rb_sb[:, :], rb_d[:, :])
            bbt_sb = constp.tile([2 * CH, 1], F32)
            nc.sync.dma_start(bbt_sb[:, :], bbt_d[:, :])
            bbs_sb = constp.tile([2 * CH, 1], F32)
            nc.sync.dma_start(bbs_sb[:, :], bbs_d[:, :])

            # Block-diagonal per-(gate,tap) kernel tiles: block q=2k+g holds
            # lhsT[64b+i, 64b+o'] = kern[b, g*64+o', i, k].
            kern_raw = kernp.tile([2 * CH, 6 * 128], BF16, name="kern_raw")
            nc.vector.memset(kern_raw[:, :].bitcast(F32), 0.0)
            kern = kernp.tile([2 * CH, 6 * 128], BF16, name="kern")
            bias_t = kernp.tile([2 * CH, 1], F32)
            bias_s = kernp.tile([2 * CH, 1], F32)
            # tanh-gate activations for all tiles (pass 1 output)
            ta_all = xpool.tile([2 * CH, T], BF16, name="ta_all")

            # ---------------- phase A: conditioned bias ---------------------
            with (
                tc.tile_pool(name="bps", bufs=1, space="PSUM") as bpsp,
                tc.tile_pool(name="bstg", bufs=1) as bstgp,
            ):
                pb = bpsp.tile([2 * CH, BL], F32)
                nc.tensor.matmul(pb[:, :], bw_sb[:, :], cT_sb[:, :], start=True, stop=True)
                pbs = bstgp.tile([2 * CH, BL], F32)
                nc.vector.tensor_copy(pbs[:, :], pb[:, :])
                # pair layout: rows 64b+o' = bias for sample b, out-chan o'
                nc.sync.dma_start(bias_t[0:CH, :], pbs[0:CH, 0:1])
                nc.sync.dma_start(bias_t[CH : 2 * CH, :], pbs[0:CH, 1:2])
                nc.sync.dma_start(bias_s[0:CH, :], pbs[CH : 2 * CH, 0:1])
                nc.sync.dma_start(bias_s[CH : 2 * CH, :], pbs[CH : 2 * CH, 1:2])
                nc.vector.tensor_add(bias_t[:, :], bias_t[:, :], bbt_sb[:, :])
                nc.vector.tensor_add(bias_s[:, :], bias_s[:, :], bbs_sb[:, :])

            # ---------------- phase A: adapter -> dynamic kernels -----------
            with (
                tc.tile_pool(name="apsum", bufs=2, space="PSUM") as apsum,
                tc.tile_pool(name="stg", bufs=3) as stgp,
            ):
                for cidx in range(6):
                    q = QLIST[cidx]
                    awt = awts[cidx]
                    for h2 in range(2):
                        ps = apsum.tile([BL, 2048], F32, tag="aps")
                        for v in range(4):
                            nc.tensor.matmul(
                                ps[:, 512 * v : 512 * (v + 1)],
                                cT_sb[:, :],
                                awt[:, 2048 * h2 + 512 * v : 2048 * h2 + 512 * (v + 1)],
                                start=True,
                                stop=True,
                            )
                        # drain PSUM with scalar and vector in parallel halves
                        stg = stgp.tile([BL, 2048], BF16, tag="stg")
                        nc.scalar.activation(stg[:, 0:1024], ps[:, 0:1024], AF.Copy)
                        nc.vector.tensor_copy(stg[:, 1024:2048], ps[:, 1024:2048])
                        for b in range(BL):
                            nc.sync.dma_start(
                                kern_raw[
                                    CH * b + 32 * h2 : CH * b + 32 * h2 + 32,
                                    128 * q + CH * b : 128 * q + CH * b + CH,
                                ],
                                stg[b : b + 1, :],
                            )
                    nc.vector.tensor_add(
                        kern[:, 128 * q : 128 * (q + 1)],
                        kern_raw[:, 128 * q : 128 * (q + 1)],
                        ab_sb[:, 128 * q : 128 * (q + 1)],
                    )

            # ---------------- phase B pass 1: tanh-gate conv ----------------
            # Only needs the even-q kernel blocks (streamed first), so it
            # overlaps the tail of the adapter weight load.
            with tc.tile_pool(name="tpsum", bufs=3, space="PSUM") as tpsum:
                for j in range(NJ):
                    pt = tpsum.tile([2 * CH, UW], F32, tag="pt")
                    for k in range(K):
                        q = 2 * k
                        for h in range(UW // NT):
                            c0 = j * UW + h * NT + DIL * k
                            nc.tensor.matmul(
                                pt[:, h * NT : (h + 1) * NT],
                                kern[:, 128 * q : 128 * (q + 1)],
                                xz[:, c0 : c0 + NT],
                                start=(k == 0),
                                stop=(k == K - 1),
                            )
                    nc.scalar.activation(
                        ta_all[:, j * UW : (j + 1) * UW],
                        pt[:, :],
                        AF.Tanh,
                        bias=bias_t[:, 0:1],
                    )

            # ---------------- phase B pass 2: sig conv + gate + residual ----
            with (
                tc.tile_pool(name="spsum", bufs=2, space="PSUM") as spsum,
                tc.tile_pool(name="opsum", bufs=2, space="PSUM") as opsum,
                tc.tile_pool(name="work", bufs=2) as workp,
            ):
                def emit_residual(j, zz):
                    po = opsum.tile([2 * CH, UW], F32, tag="po")
                    for h in range(UW // NT):
                        nc.tensor.matmul(
                            po[:, h * NT : (h + 1) * NT],
                            rwT_sb[:, :],
                            zz[:, h * NT : (h + 1) * NT],
                            start=True,
                            stop=True,
                        )
                    ot = workp.tile([2 * CH, UW], BF16, tag="ot")
                    nc.vector.scalar_tensor_tensor(
                        ot[:, :],
                        po[:, :],
                        rb_sb[:, 0:1],
                        xz[:, j * UW + PAD : j * UW + PAD + UW],
                        ALU.add,
                        ALU.add,
                    )
                    nc.sync.dma_start(out_d[:, j * UW : (j + 1) * UW], ot[:, :])

                prev = None
                for j in range(NJ):
                    psg = spsum.tile([2 * CH, UW], F32, tag="ps")
                    for k in range(K):
                        q = 2 * k + 1
                        for h in range(UW // NT):
                            c0 = j * UW + h * NT + DIL * k
                            nc.tensor.matmul(
                                psg[:, h * NT : (h + 1) * NT],
                                kern[:, 128 * q : 128 * (q + 1)],
                                xz[:, c0 : c0 + NT],
                                start=(k == 0),
                                stop=(k == K - 1),
                            )
                    # residual of previous tile: its zz is ready by now, and
                    # emitting it here keeps the PE stream dense.
                    if prev is not None:
                        emit_residual(*prev)
                    ts = workp.tile([2 * CH, UW], BF16, tag="ts")
                    nc.scalar.activation(
                        ts[:, :], psg[:, :], AF.Sigmoid, bias=bias_s[:, 0:1]
                    )
                    zz = workp.tile([2 * CH, UW], BF16, tag="zz")
                    nc.gpsimd.tensor_mul(
                        zz[:, :], ta_all[:, j * UW : (j + 1) * UW], ts[:, :]
                    )
                    nc.sync.dma_start(z_d[:, j * UW : (j + 1) * UW], zz[:, :])
                    prev = (j, zz)
                emit_residual(*prev)

    nc.compile()
    return nc


def get_nc():
    global _NC
    if _NC is None:
        _NC = _build()
    return _NC


def make_in_maps(inputs):
    import ml_dtypes

    bf = ml_dtypes.bfloat16

    x = np.asarray(inputs["x"], np.float32)
    c = np.asarray(inputs["c"], np.float32)
    aw = np.asarray(inputs["adapter_w"], np.float32)
    ab = np.asarray(inputs["adapter_b"], np.float32)
    bw = np.asarray(inputs["bias_w"], np.float32)
    bb = np.asarray(inputs["bias_b"], np.float32)
    rw = np.asarray(inputs["resi_w"], np.float32)
    rb = np.asarray(inputs["resi_b"], np.float32)

    # adapter cols [cond, (g,o',i,k)] -> chunks of (i, o') per q=2k+g in QLIST order
    aw4 = aw.reshape(COND, 2, CH, CH, K)
    chunks = []
    for cidx in range(6):
        q = QLIST[cidx]
        g, k = q % 2, q // 2
        blk = aw4[:, g, :, :, k]  # [cond, o', i]
        chunks.append(np.ascontiguousarray(blk.transpose(0, 2, 1)).reshape(COND, QCH))
    aw_r = np.ascontiguousarray(np.concatenate(chunks, axis=1).astype(bf))

    # adapter bias in the block-diagonal pair layout (zeros off-diagonal)
    ab4 = ab.reshape(2, CH, CH, K)
    ab_p = np.zeros((2 * CH, 6 * 128), np.float32)
    for q in range(6):
        g, k = q % 2, q // 2
        blk = ab4[g, :, :, k].T  # [i, o']
        for b2 in range(BL):
            ab_p[CH * b2 : CH * (b2 + 1), 128 * q + CH * b2 : 128 * q + CH * b2 + CH] = blk
    ab_p = np.ascontiguousarray(ab_p.astype(bf))

    rwT_p = np.zeros((2 * CH, 2 * CH), np.float32)
    rwT_p[0:CH, 0:CH] = rw.T
    rwT_p[CH:, CH:] = rw.T
    rwT_p = np.ascontiguousarray(rwT_p.astype(bf))

    bbt = np.ascontiguousarray(np.tile(bb[0:CH], 2).reshape(2 * CH, 1))
    bbs = np.ascontiguousarray(np.tile(bb[CH:], 2).reshape(2 * CH, 1))
    rbp = np.ascontiguousarray(np.tile(rb, 2).reshape(2 * CH, 1))
    bw_b = np.ascontiguousarray(bw.astype(bf))

    in_maps = []
    for m in range(NCORES):
        sl = slice(BL * m, BL * (m + 1))
        in_maps.append(
            {
                "x_in": np.ascontiguousarray(
                    x[sl].reshape(2 * CH, T).astype(bf)
                ),
                "cT": np.ascontiguousarray(c[sl].T.astype(bf)),
                "aw_r": aw_r,
                "ab_p": ab_p,
                "bw": bw_b,
                "bbt": bbt,
                "bbs": bbs,
                "rwT": rwT_p,
                "rb": rbp,
            }
        )
    return in_maps


def kernel(**inputs):
    global LAST_RESULTS
    nc = get_nc()
    in_maps = make_in_maps(inputs)
    res = run_bass_kernel_spmd(nc, in_maps, list(range(NCORES)), trace=TRACE)
    LAST_RESULTS = res
    out = np.empty((B, CH, T), np.float32)
    z = np.empty((B, CH, T), np.float32)
    for m in range(NCORES):
        out[BL * m : BL * (m + 1)] = (
            res.results[m]["out_d"].astype(np.float32).reshape(BL, CH, T)
        )
        z[BL * m : BL * (m + 1)] = (
            res.results[m]["z_d"].astype(np.float32).reshape(BL, CH, T)
        )
    return out, z
